# revision 1
# baseline (speedup 1.0000x reference)
"""Chamfer distance loss on 8 Trainium2 NeuronCores.

Sharding: data-parallel over batch B=8 — one Chamfer pair (4096 pred pts,
4096 gt pts, 3-D) per core; host sums the 8 per-batch scalars.

Per-core algorithm:
  The squared-distance matrix D[n,m] = ||p_n||^2 + ||g_m||^2 - 2 p.g is
  produced directly by the TensorEngine as a K=9 matmul over augmented
  embeddings (host-built, O(N) prep):
      P*[9,N] = [px,py,pz, px^2,py^2,pz^2, 1,1,1]
      G*[9,M] = [-2gx,-2gy,-2gz, 1,1,1, gx^2,gy^2,gz^2]
  so D = P*^T @ G* exactly (fp32 PSUM accumulation).
  D tiles [128n x 2048m] are consumed from PSUM by the VectorEngine:
      reduce_min over the free axis -> dist1 partials (min over gt)
      tensor_tensor min into acc    -> dist2 accumulator (min over pred)
  m-halves are processed half-by-half so each half's dist2 tail (16 PE
  transposes of acc into one PSUM slot + a single wide reduce, giving the
  partition-axis min) overlaps the other half's main loop. Ones-matmuls do
  the final partition sums; the per-batch scalar is (sum d1 + sum d2)/4096.
"""

import sys

if "/opt/trn_rl_repo" not in sys.path:
    sys.path.insert(0, "/opt/trn_rl_repo")

from contextlib import ExitStack

import numpy as np

import concourse.bacc as bacc
import concourse.mybir as mybir
import concourse.tile as tile
from concourse.bass_utils import run_bass_kernel_spmd
from concourse.masks import make_identity

B = 8
N = 4096  # pred points per batch
M = 4096  # gt points per batch
KAUG = 9
F32 = mybir.dt.float32
MIN = mybir.AluOpType.min
ADD = mybir.AluOpType.add
X = mybir.AxisListType.X
BIG = 3.0e38
HALF = 2048  # m-columns per PSUM unit (4 banks)


def _emit(ctx: ExitStack, tc: tile.TileContext, out_ap, pa_ap, ga_ap, reps=1):
    nc = tc.nc
    const_pool = ctx.enter_context(tc.tile_pool(name="const", bufs=1))
    acc_pool = ctx.enter_context(tc.tile_pool(name="acc", bufs=1))
    ps_pool = ctx.enter_context(tc.tile_pool(name="ps", bufs=2, space="PSUM"))
    small_pool = ctx.enter_context(tc.tile_pool(name="small", bufs=1))

    P = const_pool.tile([KAUG, N], F32)
    G = const_pool.tile([KAUG, M], F32)
    nc.sync.dma_start(P[:], pa_ap)
    nc.sync.dma_start(G[:], ga_ap)

    ident = const_pool.tile([128, 128], F32)
    make_identity(nc, ident[:])
    ones = const_pool.tile([128, 1], F32)
    nc.gpsimd.memset(ones[:], 1.0)

    acc = acc_pool.tile([128, M], F32)  # running min over n-tiles, per m
    rowmin = acc_pool.tile([128, 64], F32)  # per (half, n-tile) row mins
    # cols 0-31: dist2 per-128-col partition mins; cols 32-63: dist1 folds
    colmin = acc_pool.tile([128, 64], F32)

    n_tiles = N // 128

    def body(_=None):
        nc.gpsimd.memset(acc[:], BIG)
        for h in range(M // HALF):
            for i in range(n_tiles):
                lhsT = P[:, i * 128 : (i + 1) * 128]
                unit = ps_pool.tile([128, HALF], F32, tag="unit")
                for j in range(HALF // 512):
                    m0 = h * HALF + j * 512
                    nc.tensor.matmul(
                        unit[:, j * 512 : (j + 1) * 512],
                        lhsT,
                        G[:, m0 : m0 + 512],
                        start=True,
                        stop=True,
                    )
                col = h * n_tiles + i
                nc.vector.tensor_reduce(
                    rowmin[:, col : col + 1], unit[:], axis=X, op=MIN
                )
                nc.vector.tensor_tensor(
                    acc[:, h * HALF : (h + 1) * HALF],
                    unit[:],
                    acc[:, h * HALF : (h + 1) * HALF],
                    op=MIN,
                )
            # dist2 tail for this half: partition-axis min of acc via 16 PE
            # transposes into one PSUM slot + a single wide reduce.
            # Overlaps the next half's main loop.
            tp = ps_pool.tile([128, HALF], F32, tag="unit")
            for c in range(HALF // 128):
                mc = h * HALF + c * 128
                nc.tensor.transpose(
                    tp[:, c * 128 : (c + 1) * 128],
                    acc[:, mc : mc + 128],
                    ident[:],
                )
            cc = h * (HALF // 128)
            nc.vector.tensor_reduce(
                colmin[:, cc : cc + HALF // 128],
                tp[:].rearrange("p (c q) -> p c q", q=128),
                axis=X,
                op=MIN,
            )

        # ---- combine: one reduce + one partition-sum over both parts ----
        nc.vector.tensor_tensor(
            colmin[:, 32:64], rowmin[:, 0:32], rowmin[:, 32:64], op=MIN
        )
        s1 = small_pool.tile([128, 1], F32)
        nc.vector.tensor_reduce(s1[:], colmin[:], axis=X, op=ADD)
        sc = ps_pool.tile([1, 1], F32, tag="unit")
        nc.tensor.matmul(sc[:], ones[:], s1[:], start=True, stop=True)
        res = small_pool.tile([1, 1], F32)
        nc.vector.tensor_scalar(res[:], sc[:], 1.0 / N, None, op0=mybir.AluOpType.mult)
        nc.sync.dma_start(out_ap, res[:])

    if reps == 1:
        body()
    else:
        with tc.For_i(0, reps, 1) as _i:
            body(_i)


_CACHE = {}


def _build(reps=1):
    key = ("nc", reps)
    if key in _CACHE:
        return _CACHE[key]
    nc = bacc.Bacc("TRN2", target_bir_lowering=False, debug=False, num_devices=B)
    pa = nc.dram_tensor("p_aug", [KAUG, N], F32, kind="ExternalInput").ap()
    ga = nc.dram_tensor("g_aug", [KAUG, M], F32, kind="ExternalInput").ap()
    out = nc.dram_tensor("out", [1, 1], F32, kind="ExternalOutput").ap()
    with tile.TileContext(nc) as tc:
        with ExitStack() as ctx:
            _emit(ctx, tc, out, pa, ga, reps=reps)
    nc.compile()
    _CACHE[key] = nc
    return nc


def _augment(pred_b: np.ndarray, gt_b: np.ndarray):
    """pred_b: [3, N]; gt_b: [M, 3] -> (P*[9,N], G*[9,M]) fp32."""
    p = np.asarray(pred_b, dtype=np.float32)
    g = np.ascontiguousarray(np.asarray(gt_b, dtype=np.float32).T)  # [3, M]
    pa = np.empty((KAUG, N), np.float32)
    pa[0:3] = p
    pa[3:6] = p * p
    pa[6:9] = 1.0
    ga = np.empty((KAUG, M), np.float32)
    ga[0:3] = -2.0 * g
    ga[3:6] = 1.0
    ga[6:9] = g * g
    return pa, ga


def _make_runner(nc):
    """Persistent jit over the bass_exec custom call — identical execution
    path to run_bass_kernel_spmd under axon, but the jitted callable is
    cached so repeat kernel() calls skip ~180ms of retracing."""
    import jax
    from jax.sharding import Mesh, PartitionSpec
    from jax.experimental.shard_map import shard_map

    from concourse.bass2jax import (
        _bass_exec_p,
        install_neuronx_cc_hook,
        partition_id_tensor,
    )

    install_neuronx_cc_hook()
    partition_name = nc.partition_id_tensor.name if nc.partition_id_tensor else None
    in_names, out_names, out_avals, zero_outs = [], [], [], []
    for alloc in nc.m.functions[0].allocations:
        if not isinstance(alloc, mybir.MemoryLocationSet):
            continue
        name = alloc.memorylocations[0].name
        if alloc.kind == "ExternalInput":
            if name != partition_name:
                in_names.append(name)
        elif alloc.kind == "ExternalOutput":
            shape = tuple(alloc.tensor_shape)
            dtype = mybir.dt.np(alloc.dtype)
            out_names.append(name)
            out_avals.append(jax.core.ShapedArray(shape, dtype))
            zero_outs.append(np.zeros(shape, dtype))
    n_params = len(in_names)
    all_in_names = list(in_names) + list(out_names)
    if partition_name is not None:
        all_in_names.append(partition_name)

    def _body(*args):
        operands = list(args)
        if partition_name is not None:
            operands.append(partition_id_tensor())
        return tuple(
            _bass_exec_p.bind(
                *operands,
                out_avals=tuple(out_avals),
                in_names=tuple(all_in_names),
                out_names=tuple(out_names),
                lowering_input_output_aliases=(),
                sim_require_finite=True,
                sim_require_nnan=True,
                nc=nc,
            )
        )

    devices = jax.devices()[:B]
    mesh = Mesh(np.asarray(devices), ("core",))
    n_outs = len(out_avals)
    fn = jax.jit(
        shard_map(
            _body,
            mesh=mesh,
            in_specs=(PartitionSpec("core"),) * (n_params + n_outs),
            out_specs=(PartitionSpec("core"),) * n_outs,
            check_rep=False,
        ),
        donate_argnums=tuple(range(n_params, n_params + n_outs)),
        keep_unused=True,
    )

    def run(in_maps):
        concat_in = [
            np.concatenate([np.asarray(in_maps[c][nm]) for c in range(B)], axis=0)
            for nm in in_names
        ]
        concat_zeros = [
            np.zeros((B * z.shape[0], *z.shape[1:]), z.dtype) for z in zero_outs
        ]
        outs = [np.asarray(o) for o in fn(*concat_in, *concat_zeros)]
        return [
            {nm: outs[i].reshape(B, *out_avals[i].shape)[c] for i, nm in enumerate(out_names)}
            for c in range(B)
        ]

    return run


def kernel(pred_pts: np.ndarray, gt_pts: np.ndarray) -> np.ndarray:
    pred_pts = np.asarray(pred_pts)
    gt_pts = np.asarray(gt_pts)
    nc = _build()
    in_maps = []
    for b in range(B):
        pa, ga = _augment(pred_pts[b], gt_pts[b])
        in_maps.append({"p_aug": pa, "g_aug": ga})
    if "runner" not in _CACHE:
        # First call goes through the stock entry point (compiles the NEFF);
        # cache a persistent runner for subsequent calls.
        results = run_bass_kernel_spmd(nc, in_maps, core_ids=list(range(B))).results
        try:
            _CACHE["runner"] = _make_runner(nc)
        except Exception:
            pass
    else:
        results = _CACHE["runner"](in_maps)
    per_batch = np.array(
        [results[b]["out"].reshape(()) for b in range(B)], dtype=np.float32
    )
    return np.asarray(per_batch.sum() / np.float32(B), dtype=np.float32)



# revision 11
# speedup vs baseline: 2.1343x; 2.1343x over previous
"""Chamfer distance loss on 8 Trainium2 NeuronCores.

Sharding: data-parallel over batch B=8 - one Chamfer pair (4096 pred pts,
4096 gt pts, 3-D) per core; host sums the 8 per-batch scalars.

Per-core algorithm (max-form: the PE emits -D so every min becomes a max):
  -D[n,m] = 2 p.g - ||p||^2 - ||g||^2 is produced by the TensorEngine as a
  K=21 bf16 matmul over hi/lo-split augmented embeddings (error ~2^-18):
      P21 = [P9hi; P9hi[cross,ones]; P9lo[lin,sq]]
      G21 = [G9hi; G9lo[cross,sq];   G9hi[lin,sq]]  (G9 pre-negated)
  Work unit = [128 pred rows x 2048 gt cols] fp32 in PSUM (2 rotating bufs).
  Per unit the VectorEngine runs ONE fused tensor_tensor_reduce over the
  unit's two 1024-col halves: accum = exact per-row max (dist1!) while
  reading each PSUM element only once per purpose. For dist2 the unit is
  converted to bf16 (ScalarEngine copy, or a DVE pair-fold of two n-tiles
  straight from PSUM) and chained into a running column max `acc` that is
  split into a DVE stripe (bf16 2x) and a GpSimd stripe. The acc
  partition-axis max uses gpsimd.partition_all_reduce for one m-half and
  PE transposes + one reduce for the other. Ones-matmul sums finish.
"""

import sys

if "/opt/trn_rl_repo" not in sys.path:
    sys.path.insert(0, "/opt/trn_rl_repo")

from contextlib import ExitStack

import numpy as np

import concourse.bacc as bacc
import concourse.bass_isa as bass_isa
import concourse.mybir as mybir
import concourse.tile as tile
from concourse.bass_utils import run_bass_kernel_spmd
from concourse.masks import make_identity

B = 8
N = 4096  # pred points per batch
M = 4096  # gt points per batch
KAUG = 21
F32 = mybir.dt.float32
BF16 = mybir.dt.bfloat16
MAX = mybir.AluOpType.max
ADD = mybir.AluOpType.add
X = mybir.AxisListType.X
NEG = -3.0e38
UW = 2048  # unit width (m-cols per PSUM unit)
TTR_TILES = ()  # tiles converted by DVE TTR-copy instead of Act


def _emit(ctx: ExitStack, tc: tile.TileContext, out_ap, pa_ap, ga_ap, reps=1):
    nc = tc.nc
    const_pool = ctx.enter_context(tc.tile_pool(name="const", bufs=1))
    acc_pool = ctx.enter_context(tc.tile_pool(name="acc", bufs=1))
    db_pool = ctx.enter_context(tc.tile_pool(name="db", bufs=3))
    small_pool = ctx.enter_context(tc.tile_pool(name="small", bufs=1))

    P = const_pool.tile([KAUG, N], BF16)
    G = const_pool.tile([KAUG, M], BF16)
    nc.sync.dma_start(P[:], pa_ap)
    nc.sync.dma_start(G[:], ga_ap)

    ident = const_pool.tile([128, 128], BF16)
    make_identity(nc, ident[:])
    ones = const_pool.tile([128, 1], F32)
    nc.gpsimd.memset(ones[:], 1.0)

    # acc[p, m] = max over processed n-tiles of -D[., m]
    acc = acc_pool.tile([128, M], BF16)
    # rr cols: [0,32) per-tile row maxes (-dist1); [32,64) transposed col maxes
    rr = acc_pool.tile([128, 64], F32)
    scr1 = acc_pool.tile([128, 2048], BF16)
    scr2 = acc_pool.tile([128, 1024], BF16)
    scr3 = acc_pool.tile([128, 2048], BF16)  # 4 tiles x 512 fold3 outputs

    n_tiles = N // 128

    def body(_=None):
        with tc.tile_pool(name="ps", bufs=2, space="PSUM") as ps_pool:
            for i in range(n_tiles):
                lhsT = P[:, i * 128 : (i + 1) * 128]
                if i == 0:
                    db = acc  # first tile initializes acc via its own copy
                else:
                    db = db_pool.tile([128, M], BF16, tag="db")
                for h in range(2):
                    u = ps_pool.tile([128, UW], F32, tag="u")
                    for j in range(UW // 512):
                        m0 = h * UW + j * 512
                        nc.tensor.matmul(
                            u[:, j * 512 : (j + 1) * 512],
                            lhsT,
                            G[:, m0 : m0 + 512],
                            start=True,
                            stop=True,
                        )
                    nc.scalar.activation(
                        db[:, h * UW : (h + 1) * UW],
                        u[:],
                        mybir.ActivationFunctionType.Copy,
                    )
                # row-max fold tree (bf16, 2x DVE): 4096 -> 2048 -> 1024 -> 512
                nc.vector.tensor_tensor(scr1[:], db[:, 0:2048], db[:, 2048:4096], op=MAX)
                nc.vector.tensor_tensor(scr2[:], scr1[:, 0:1024], scr1[:, 1024:2048], op=MAX)
                s3 = scr3[:, (i % 4) * 512 : (i % 4 + 1) * 512]
                nc.vector.tensor_tensor(s3, scr2[:, 0:512], scr2[:, 512:1024], op=MAX)
                if i % 4 == 3:
                    # batched reduce: one TR yields 4 tiles' row maxes
                    nc.vector.tensor_reduce(
                        rr[:, i - 3 : i + 1],
                        scr3[:].rearrange("p (c q) -> p c q", q=512),
                        axis=X,
                        op=MAX,
                    )
                if i > 0:
                    nc.vector.tensor_tensor(acc[:], db[:], acc[:], op=MAX)

        # ---- tails ----
        # dist2: PE transposes + one strided reduce per m-half
        with tc.tile_pool(name="pst", bufs=2, space="PSUM") as pst_pool:
            for hh in range(2):
                tp = pst_pool.tile([128, UW], BF16, tag="tp")
                for c in range(UW // 128):
                    nc.tensor.transpose(
                        tp[:, c * 128 : (c + 1) * 128],
                        acc[:, hh * UW + c * 128 : hh * UW + (c + 1) * 128],
                        ident[:],
                    )
                nc.vector.tensor_reduce(
                    rr[:, 32 + 16 * hh : 48 + 16 * hh],
                    tp[:].rearrange("p (c q) -> p c q", q=128),
                    axis=X,
                    op=MAX,
                )

            # sums: rows of rr (dist1 maxes + transposed dist2 col maxes)
            s1 = small_pool.tile([128, 1], F32)
            nc.vector.tensor_reduce(s1[:], rr[:], axis=X, op=ADD)
            sc = pst_pool.tile([1, 2], F32)
            nc.tensor.matmul(sc[:, 0:1], ones[:], s1[:], start=True, stop=True)
            res = small_pool.tile([1, 1], F32)
            nc.vector.tensor_scalar(
                res[:], sc[:, 0:1], -1.0 / N, None, op0=mybir.AluOpType.mult
            )
            nc.sync.dma_start(out_ap, res[:])

    if reps == 1:
        body()
    else:
        with tc.For_i(0, reps, 1) as _i:
            body(_i)


_CACHE = {}


def _build(reps=1):
    key = ("nc", reps)
    if key in _CACHE:
        return _CACHE[key]
    nc = bacc.Bacc("TRN2", target_bir_lowering=False, debug=False, num_devices=B)
    pa = nc.dram_tensor("p_aug", [KAUG, N], BF16, kind="ExternalInput").ap()
    ga = nc.dram_tensor("g_aug", [KAUG, M], BF16, kind="ExternalInput").ap()
    out = nc.dram_tensor("out", [1, 1], F32, kind="ExternalOutput").ap()
    with tile.TileContext(nc) as tc:
        with ExitStack() as ctx:
            _emit(ctx, tc, out, pa, ga, reps=reps)
    nc.compile()
    _CACHE[key] = nc
    return nc


def _augment(pred_b: np.ndarray, gt_b: np.ndarray):
    """pred_b: [3, N]; gt_b: [M, 3] -> (P21[21,N], G21[21,M]) bf16.

    Max-form hi/lo split: sum_k P21[k] G21[k] = -D to ~2^-18 relative.
    """
    import ml_dtypes

    bf = ml_dtypes.bfloat16
    p = np.asarray(pred_b, dtype=np.float32)  # [3, N]
    g = np.ascontiguousarray(np.asarray(gt_b, dtype=np.float32).T)  # [3, M]

    p9 = np.empty((9, N), np.float32)
    p9[0:3] = p
    p9[3:6] = p * p
    p9[6:9] = 1.0
    g9 = np.empty((9, M), np.float32)
    g9[0:3] = 2.0 * g
    g9[3:6] = -1.0
    g9[6:9] = -(g * g)

    p9h = p9.astype(bf)
    p9l = (p9 - p9h.astype(np.float32)).astype(bf)
    g9h = g9.astype(bf)
    g9l = (g9 - g9h.astype(np.float32)).astype(bf)

    cross_sq = [0, 1, 2, 6, 7, 8]  # G rows with nonzero lo
    lin_sq = [0, 1, 2, 3, 4, 5]  # P rows with nonzero lo
    p21 = np.concatenate([p9h, p9h[cross_sq], p9l[lin_sq]], axis=0)
    g21 = np.concatenate([g9h, g9l[cross_sq], g9h[lin_sq]], axis=0)
    return p21, g21


def _make_runner(nc):
    """Persistent jit over the bass_exec custom call — identical execution
    path to run_bass_kernel_spmd under axon, but the jitted callable is
    cached so repeat kernel() calls skip ~180ms of retracing."""
    import jax
    from jax.sharding import Mesh, PartitionSpec
    from jax.experimental.shard_map import shard_map

    from concourse.bass2jax import (
        _bass_exec_p,
        install_neuronx_cc_hook,
        partition_id_tensor,
    )

    install_neuronx_cc_hook()
    partition_name = nc.partition_id_tensor.name if nc.partition_id_tensor else None
    in_names, out_names, out_avals, zero_outs = [], [], [], []
    for alloc in nc.m.functions[0].allocations:
        if not isinstance(alloc, mybir.MemoryLocationSet):
            continue
        name = alloc.memorylocations[0].name
        if alloc.kind == "ExternalInput":
            if name != partition_name:
                in_names.append(name)
        elif alloc.kind == "ExternalOutput":
            shape = tuple(alloc.tensor_shape)
            dtype = mybir.dt.np(alloc.dtype)
            out_names.append(name)
            out_avals.append(jax.core.ShapedArray(shape, dtype))
            zero_outs.append(np.zeros(shape, dtype))
    n_params = len(in_names)
    all_in_names = list(in_names) + list(out_names)
    if partition_name is not None:
        all_in_names.append(partition_name)

    def _body(*args):
        operands = list(args)
        if partition_name is not None:
            operands.append(partition_id_tensor())
        return tuple(
            _bass_exec_p.bind(
                *operands,
                out_avals=tuple(out_avals),
                in_names=tuple(all_in_names),
                out_names=tuple(out_names),
                lowering_input_output_aliases=(),
                sim_require_finite=True,
                sim_require_nnan=True,
                nc=nc,
            )
        )

    devices = jax.devices()[:B]
    mesh = Mesh(np.asarray(devices), ("core",))
    n_outs = len(out_avals)
    fn = jax.jit(
        shard_map(
            _body,
            mesh=mesh,
            in_specs=(PartitionSpec("core"),) * (n_params + n_outs),
            out_specs=(PartitionSpec("core"),) * n_outs,
            check_rep=False,
        ),
        donate_argnums=tuple(range(n_params, n_params + n_outs)),
        keep_unused=True,
    )

    def run(in_maps):
        concat_in = [
            np.concatenate([np.asarray(in_maps[c][nm]) for c in range(B)], axis=0)
            for nm in in_names
        ]
        concat_zeros = [
            np.zeros((B * z.shape[0], *z.shape[1:]), z.dtype) for z in zero_outs
        ]
        outs = [np.asarray(o) for o in fn(*concat_in, *concat_zeros)]
        return [
            {nm: outs[i].reshape(B, *out_avals[i].shape)[c] for i, nm in enumerate(out_names)}
            for c in range(B)
        ]

    return run


def kernel(pred_pts: np.ndarray, gt_pts: np.ndarray) -> np.ndarray:
    pred_pts = np.asarray(pred_pts)
    gt_pts = np.asarray(gt_pts)
    nc = _build()
    in_maps = []
    for b in range(B):
        pa, ga = _augment(pred_pts[b], gt_pts[b])
        in_maps.append({"p_aug": pa, "g_aug": ga})
    if "runner" not in _CACHE:
        # First call goes through the stock entry point (compiles the NEFF);
        # cache a persistent runner for subsequent calls.
        results = run_bass_kernel_spmd(nc, in_maps, core_ids=list(range(B))).results
        try:
            _CACHE["runner"] = _make_runner(nc)
        except Exception:
            pass
    else:
        results = _CACHE["runner"](in_maps)
    per_batch = np.array(
        [results[b]["out"].reshape(()) for b in range(B)], dtype=np.float32
    )
    return np.asarray(per_batch.sum() / np.float32(B), dtype=np.float32)
